# revision 5
# baseline (speedup 1.0000x reference)
"""GrwSmoothingLoss on 8 Trainium2 NeuronCores.

Math: for each batch b, with Gram matrix G_b = Z_b @ Z_b^T (8x8),
  logits[b,p] = -0.5 * ||diff2(Z_b[perm_p])||^2 = -0.5 * <C_p, G_b>,
  where C_p = M_p^T M_p and M_p is the 6x8 permuted second-difference matrix.
The smoothness term is also linear in G_b:  V_b = 0.5 * <C1, G_b>.

All C_p entries are small integers in [-4, 6], so the whole 64 x 1025
coefficient table is built on the host and shipped exactly in fp8e4.
Per core (32 batches):
  1. cross-Gram via two PE matmuls on zt[k,(b,t)] (bf16),
  2. diagonal-block extraction: mask-multiply + strided reduce (DVE),
     then an expand + two selection matmuls to get gT[(i,j), b],
  3. eight perm-major matmuls X[p', b] (fp8 stationary x bf16 moving),
  4. one Exp pass over [128, 256] (logits <= 0 so no max-subtraction),
  5. ones-matmul column sum -> per-(chunk,b) partial sums,
  6. DMA out one [1, 320] row: 256 partials + X0 row + alpha*V row.
Host finishes with ln / means over 256 batches (trivial postprocess).

Sharding: data-parallel over B (32 batches/core); coefficient table
replicated.
"""

import numpy as np
import ml_dtypes

import concourse.bacc as bacc
import concourse.bass as bass
import concourse.mybir as mybir
import concourse.tile as tile
from concourse.bass_utils import run_bass_kernel_spmd

B, T, K = 256, 8, 128
NUM_PERMS = 1000
PPAD = 1024  # perm columns padded to 8 chunks of 128
ALPHA = 0.5
N_CORES = 8
B_LOC = B // N_CORES
F32 = mybir.dt.float32
BF16 = mybir.dt.bfloat16
FP8 = mybir.dt.float8e4

_cache = {}


def _consts():
    # consts pack [128, 577] bf16: mask2 | maskI | ones
    p = np.arange(128)
    b1 = p >> 3  # batch-within-half index of each psum partition
    i8 = p & 7
    # mask2[p, h*256 + b2*8 + j] = (b2 == b1 + 16h)
    mask2 = np.zeros((128, 2, 32, 8), np.float32)
    for h in range(2):
        mask2[p, h, b1 + 16 * h, :] = 1.0
    # maskI[p, 8i+j] = (i == p%8)
    maskI = np.zeros((128, 8, 8), np.float32)
    maskI[p, i8, :] = 1.0
    ones = np.ones((128, 1), np.float32)
    pack = np.concatenate(
        [mask2.reshape(128, 512), maskI.reshape(128, 64), ones], axis=1
    )
    return pack.astype(ml_dtypes.bfloat16)


def _c_table(perm):
    # [64, 1025] fp8: columns 0..999 = C_p (all entries in [-4, 6]),
    # 1000..1023 pad with 240*e_0 (X_pad ~= 240 -> exp(-120) == 0),
    # 1024 = 0.5*alpha*C1 (the smoothness column).
    oh = np.eye(8, dtype=np.float32)[perm]  # [P, T, 8]
    M = oh[:, 0:6] - 2.0 * oh[:, 1:7] + oh[:, 2:8]  # [P, 6, 8]
    C = np.einsum("pri,prj->pij", M, M)  # [P, 8, 8]
    tab = np.zeros((64, PPAD + 1), np.float32)
    tab[:, :NUM_PERMS] = C.reshape(NUM_PERMS, 64).T
    tab[0, NUM_PERMS:PPAD] = 240.0
    D1 = (np.eye(T, k=1) - np.eye(T))[: T - 1]
    C1 = D1.T @ D1
    tab[:, PPAD] = 0.5 * ALPHA * C1.reshape(64)
    return tab.astype(ml_dtypes.float8_e4m3)


def _kernel_body(tc, out_d, zt_d, cpack_d, cfp8_d):
    nc = tc.nc
    with (
        tc.tile_pool(name="sb", bufs=1) as sb,
        tc.tile_pool(name="ps", bufs=1, space="PSUM") as ps,
    ):
        zt = sb.tile([128, 256], BF16)
        cpack = sb.tile([128, 577], BF16)
        cfp8 = sb.tile([64, PPAD + 1], FP8)
        nc.sync.dma_start(out=zt[:], in_=zt_d[:])
        nc.gpsimd.dma_start(out=cpack[:], in_=cpack_d[:])
        nc.gpsimd.dma_start(out=cfp8[:], in_=cfp8_d[:])
        mask2 = cpack[:, 0:512]
        maskI = cpack[:, 512:576]
        onescol = cpack[:, 576:577]
        # q16[p, b'] = (b' == p>>3): the j=0,h=0 stripe of mask2
        q16 = mask2.rearrange("p (h b j) -> p h b j", h=2, b=32)[:, 0, 0:16, 0]

        # cross-Gram: psum_cr[(b1',i), (h,b2,j)] = <Z[16h+b1',i,:], Z[b2,j,:]>
        psum_cr = ps.tile([128, 512], F32)
        nc.tensor.matmul(psum_cr[:, 0:256], zt[:, 0:128], zt[:])
        nc.tensor.matmul(psum_cr[:, 256:512], zt[:, 128:256], zt[:])

        # diagonal blocks: tmp = psum_cr * mask2 ; red[p,(h,j)] = sum_b2 tmp
        tmp = sb.tile([128, 512], F32)
        nc.vector.tensor_tensor(
            out=tmp[:], in0=psum_cr[:], in1=mask2, op=mybir.AluOpType.mult
        )
        red = sb.tile([128, 16], F32)
        nc.vector.tensor_reduce(
            out=red[:],
            in_=tmp[:].rearrange("p (h b j) -> p h j b", h=2, b=32),
            axis=mybir.AxisListType.X,
            op=mybir.AluOpType.add,
        )
        # Gexp[(b1',i'), (h,i,j)] = red[(b1',i'), (h,j)] * (i == i')
        gexp = sb.tile([128, 128], BF16)
        gexpv = gexp[:].rearrange("p (h i j) -> p h i j", h=2, i=8)
        red_b = (
            red[:]
            .rearrange("p (h j) -> p h j", h=2)
            .unsqueeze(2)
            .broadcast_to([128, 2, 8, 8])
        )
        mi_b = (
            maskI.rearrange("p (i j) -> p i j", i=8)
            .unsqueeze(1)
            .broadcast_to([128, 2, 8, 8])
        )
        nc.vector.tensor_tensor(out=gexpv, in0=red_b, in1=mi_b, op=mybir.AluOpType.mult)
        # gT[(i,j), 16h+b'] via selection matmuls
        psum_gt = ps.tile([64, B_LOC], F32)
        nc.tensor.matmul(psum_gt[:, 0:16], gexp[:, 0:64], q16)
        nc.tensor.matmul(psum_gt[:, 16:32], gexp[:, 64:128], q16)
        gT = sb.tile([64, B_LOC], BF16)
        nc.vector.tensor_copy(gT[:], psum_gt[:])

        # X[p', b] chunks: psum_X[p', 32c+b] = <C_{128c+p'}, G_b>
        # col 256:288 of row 0 holds the alpha*V row from the C1 column.
        psum_X = ps.tile([128, 288], F32)
        nc.tensor.matmul(psum_X[0:1, 256:288], cfp8[:, PPAD : PPAD + 1], gT[:])
        for c in range(8):
            nc.tensor.matmul(
                psum_X[:, 32 * c : 32 * (c + 1)],
                cfp8[:, 128 * c : 128 * (c + 1)],
                gT[:],
            )

        # e = exp(-0.5 X) (logits <= 0: no overflow; pads underflow to 0)
        e = sb.tile([128, 256], BF16)
        nc.scalar.activation(
            e[:], psum_X[:, 0:256], mybir.ActivationFunctionType.Exp, scale=-0.5
        )
        out_sb = sb.tile([1, 320], F32)
        # column sums on gpsimd straight into the out row:
        # s[(c,b)] = sum_p' e[p', (c,b)]
        nc.gpsimd.tensor_reduce(
            out=out_sb[:, 0:256],
            in_=e[:],
            axis=mybir.AxisListType.C,
            op=mybir.AluOpType.add,
        )
        # X0 row (cols 0:32) and alpha*V row (cols 256:288) in one strided copy
        x0v = psum_X[0:1, 0:288].rearrange("p (a b) -> p a b", a=9)[:, 0:9:8, :]
        nc.vector.tensor_copy(out_sb[:, 256:320].rearrange("p (a b) -> p a b", a=2), x0v)
        nc.sync.dma_start(out=out_d[:], in_=out_sb[:])


def _build():
    if "nc" in _cache:
        return _cache["nc"]
    nc = bacc.Bacc(
        "TRN2",
        target_bir_lowering=False,
        debug=False,
        enable_asserts=False,
        num_devices=N_CORES,
    )
    zt_d = nc.dram_tensor("zt", [128, 256], BF16, kind="ExternalInput").ap()
    cpack_d = nc.dram_tensor("cpack", [128, 577], BF16, kind="ExternalInput").ap()
    cfp8_d = nc.dram_tensor("cfp8", [64, PPAD + 1], FP8, kind="ExternalInput").ap()
    out_d = nc.dram_tensor("out_row", [1, 320], F32, kind="ExternalOutput").ap()
    with tile.TileContext(nc) as tc:
        _kernel_body(tc, out_d, zt_d, cpack_d, cfp8_d)
    nc.compile()
    _cache["nc"] = nc
    return nc


def _in_maps(Z, perm_index):
    perm = np.asarray(perm_index, dtype=np.int64).reshape(NUM_PERMS, T)
    key = perm.tobytes()
    if _cache.get("ckey") != key:
        _cache["ckey"] = key
        _cache["ctab"] = _c_table(perm)
        _cache["cpack"] = _consts()
    ctab, cpack = _cache["ctab"], _cache["cpack"]
    Zf = np.asarray(Z, dtype=np.float32)
    in_maps = []
    for c in range(N_CORES):
        zc = Zf[c * B_LOC : (c + 1) * B_LOC]  # [32, 8, 128]
        zt = np.ascontiguousarray(zc.transpose(2, 0, 1).reshape(128, 256))
        in_maps.append(
            {
                "zt": zt.astype(ml_dtypes.bfloat16),
                "cpack": cpack,
                "cfp8": ctab,
            }
        )
    return in_maps


def kernel(Z, perm_index, _trace=False):
    nc = _build()
    in_maps = _in_maps(Z, perm_index)
    res = run_bass_kernel_spmd(
        nc, in_maps, core_ids=list(range(N_CORES)), trace=_trace
    )
    total = np.float64(0.0)
    for r in res.results:
        row = np.asarray(r["out_row"], dtype=np.float64).reshape(320)
        s = row[0:256].reshape(8, B_LOC).sum(axis=0)  # [32] per-batch exp sums
        x0 = row[256:288]
        av = row[288:320]
        total += float(np.sum(np.log(s) + 0.5 * x0 + av))
    out = np.array(total / B, dtype=np.float32)
    if _trace:
        return out, res
    return out


# revision 11
# speedup vs baseline: 178433.3076x; 178433.3076x over previous
"""GrwSmoothingLoss on 8 Trainium2 NeuronCores.

Math: for each batch b, with Gram matrix G_b = Z_b @ Z_b^T (8x8),
  logits[b,p] = -0.5 * ||diff2(Z_b[perm_p])||^2 = -0.5 * <C_p, G_b>,
  where C_p = M_p^T M_p and M_p is the 6x8 permuted second-difference matrix.
The smoothness term is also linear in G_b:  V_b = 0.5 * <C1, G_b>.

All C_p entries are small integers in [-4, 6], so the whole 64 x 1025
coefficient table is built on the host and shipped exactly in fp8e4.
Per core (32 batches):
  1. cross-Gram via two PE matmuls on zt[k,(b,t)] (bf16),
  2. diagonal-block extraction: mask-multiply + strided reduce (DVE),
     then an expand + two selection matmuls to get gT[(i,j), b],
  3. eight perm-major matmuls X[p', b] (fp8 stationary x bf16 moving),
  4. one Exp pass over [128, 256] (logits <= 0 so no max-subtraction),
  5. ones-matmul column sum -> per-(chunk,b) partial sums,
  6. DMA out one [1, 320] row: 256 partials + X0 row + alpha*V row.
Host finishes with ln / means over 256 batches (trivial postprocess).

Sharding: data-parallel over B (32 batches/core); coefficient table
replicated.
"""

import numpy as np
import ml_dtypes

import concourse.bacc as bacc
import concourse.bass as bass
import concourse.mybir as mybir
import concourse.tile as tile
from concourse.bass_utils import run_bass_kernel_spmd

B, T, K = 256, 8, 128
NUM_PERMS = 1000
PPAD = 1024  # perm columns padded to 8 chunks of 128
ALPHA = 0.5
N_CORES = 8
B_LOC = B // N_CORES
F32 = mybir.dt.float32
BF16 = mybir.dt.bfloat16
FP8 = mybir.dt.float8e4

_cache = {}


def _consts():
    # consts pack [128, 193] bf16: mask_sm | maskI | ones
    p = np.arange(128)
    b1 = p >> 3  # batch-within-half index of each psum partition
    i8 = p & 7
    # mask_sm[p, b2*8 + j] = (b2 == b1), b2 in [0,16) (same for both halves)
    mask_sm = np.zeros((128, 16, 8), np.float32)
    mask_sm[p, b1, :] = 1.0
    # maskI[p, 8i+j] = (i == p%8)
    maskI = np.zeros((128, 8, 8), np.float32)
    maskI[p, i8, :] = 1.0
    ones = np.ones((128, 1), np.float32)
    pack = np.concatenate(
        [mask_sm.reshape(128, 128), maskI.reshape(128, 64), ones], axis=1
    )
    return pack.astype(ml_dtypes.bfloat16)


def _c_table(perm):
    # [64, 1025] fp8: columns 0..999 = C_p (all entries in [-4, 6]),
    # 1000..1023 pad with 240*e_0 (X_pad ~= 240 -> exp(-120) == 0),
    # 1024 = 0.5*alpha*C1 (the smoothness column).
    oh = np.eye(8, dtype=np.float32)[perm]  # [P, T, 8]
    M = oh[:, 0:6] - 2.0 * oh[:, 1:7] + oh[:, 2:8]  # [P, 6, 8]
    C = np.einsum("pri,prj->pij", M, M)  # [P, 8, 8]
    tab = np.zeros((64, PPAD + 1), np.float32)
    tab[:, :NUM_PERMS] = C.reshape(NUM_PERMS, 64).T
    tab[0, NUM_PERMS:PPAD] = 240.0
    D1 = (np.eye(T, k=1) - np.eye(T))[: T - 1]
    C1 = D1.T @ D1
    tab[:, PPAD] = 0.5 * ALPHA * C1.reshape(64)
    return tab.astype(ml_dtypes.float8_e4m3)


def _kernel_body(tc, out_d, zt_d, cpack_d, cfp8_d, loop_k=None):
    import contextlib

    nc = tc.nc
    with (
        tc.tile_pool(name="sb", bufs=1) as sb,
        tc.tile_pool(name="ps", bufs=1, space="PSUM") as ps,
        tc.For_i(0, loop_k) if loop_k else contextlib.nullcontext(),
    ):
        zt = sb.tile([128, 256], BF16)
        cpack = sb.tile([128, 193], BF16)
        cfp8 = sb.tile([64, PPAD + 1], FP8)
        nc.sync.dma_start(out=zt[:], in_=zt_d[:])
        nc.gpsimd.dma_start(out=cpack[:], in_=cpack_d[:])
        nc.gpsimd.dma_start(out=cfp8[:], in_=cfp8_d[:])
        mask_sm = cpack[:, 0:128]
        maskI = cpack[:, 128:192]
        onescol = cpack[:, 192:193]
        # q16[p, b'] = (b' == p>>3): the j=0 stripe of mask_sm
        q16 = mask_sm.rearrange("p (b j) -> p b j", b=16)[:, :, 0]

        # cross-Gram within each 16-batch half: the diagonal blocks live
        # inside the half, so the moving operand only needs its own columns.
        # psum_cr[(b1',i), (h,b2',j)] = <Z[16h+b1',i,:], Z[16h+b2',j,:]>
        psum_cr = ps.tile([128, 256], F32)
        nc.tensor.matmul(psum_cr[:, 0:128], zt[:, 0:128], zt[:, 0:128])
        nc.tensor.matmul(psum_cr[:, 128:256], zt[:, 128:256], zt[:, 128:256])

        # diagonal blocks: tmp = psum_cr * mask ; red[p,(h,j)] = sum_b2' tmp
        tmp = sb.tile([128, 256], F32)
        nc.vector.tensor_tensor(
            out=tmp[:].rearrange("p (h f) -> p h f", h=2),
            in0=psum_cr[:].rearrange("p (h f) -> p h f", h=2),
            in1=mask_sm.unsqueeze(1).broadcast_to([128, 2, 128]),
            op=mybir.AluOpType.mult,
        )
        red = sb.tile([128, 16], F32)
        nc.vector.tensor_reduce(
            out=red[:],
            in_=tmp[:].rearrange("p (h b j) -> p h j b", h=2, b=16),
            axis=mybir.AxisListType.X,
            op=mybir.AluOpType.add,
        )
        # Gexp[(b1',i'), (h,i,j)] = red[(b1',i'), (h,j)] * (i == i')
        gexp = sb.tile([128, 128], BF16)
        gexpv = gexp[:].rearrange("p (h i j) -> p h i j", h=2, i=8)
        red_b = (
            red[:]
            .rearrange("p (h j) -> p h j", h=2)
            .unsqueeze(2)
            .broadcast_to([128, 2, 8, 8])
        )
        mi_b = (
            maskI.rearrange("p (i j) -> p i j", i=8)
            .unsqueeze(1)
            .broadcast_to([128, 2, 8, 8])
        )
        nc.vector.tensor_tensor(out=gexpv, in0=red_b, in1=mi_b, op=mybir.AluOpType.mult)
        # gT[(i,j), 16h+b'] via selection matmuls
        psum_gt = ps.tile([64, B_LOC], F32)
        nc.tensor.matmul(psum_gt[:, 0:16], gexp[:, 0:64], q16)
        nc.tensor.matmul(psum_gt[:, 16:32], gexp[:, 64:128], q16)
        gT = sb.tile([64, B_LOC], BF16)
        nc.vector.tensor_copy(gT[:], psum_gt[:])

        # X[p', b] chunks: psum_X[p', 32c+b] = <C_{128c+p'}, G_b>
        # col 256:288 of row 0 holds the alpha*V row from the C1 column.
        psum_X = ps.tile([128, 288], F32)
        nc.tensor.matmul(psum_X[0:1, 256:288], cfp8[:, PPAD : PPAD + 1], gT[:])
        for c in range(8):
            nc.tensor.matmul(
                psum_X[:, 32 * c : 32 * (c + 1)],
                cfp8[:, 128 * c : 128 * (c + 1)],
                gT[:],
            )

        # e = exp(-0.5 X) (logits <= 0: no overflow; pads underflow to 0)
        e = sb.tile([128, 256], BF16)
        nc.scalar.activation(
            e[:], psum_X[:, 0:256], mybir.ActivationFunctionType.Exp, scale=-0.5
        )
        out_sb = sb.tile([1, 320], F32)
        # column sums: s[(c,b)] = sum_p' e[p', (c,b)]
        psum_s = ps.tile([1, 256], F32)
        nc.tensor.matmul(psum_s[:], onescol, e[:])
        nc.scalar.copy(out_sb[:, 0:256], psum_s[:])
        # X0 row (cols 0:32) and alpha*V row (cols 256:288) in one strided copy
        x0v = psum_X[0:1, 0:288].rearrange("p (a b) -> p a b", a=9)[:, 0:9:8, :]
        nc.vector.tensor_copy(out_sb[:, 256:320].rearrange("p (a b) -> p a b", a=2), x0v)
        nc.sync.dma_start(out=out_d[:], in_=out_sb[:])


def _build(loop_k=None):
    ckey = ("nc", loop_k)
    if ckey in _cache:
        return _cache[ckey]
    nc = bacc.Bacc(
        "TRN2",
        target_bir_lowering=False,
        debug=False,
        enable_asserts=False,
        num_devices=N_CORES,
    )
    zt_d = nc.dram_tensor("zt", [128, 256], BF16, kind="ExternalInput").ap()
    cpack_d = nc.dram_tensor("cpack", [128, 193], BF16, kind="ExternalInput").ap()
    cfp8_d = nc.dram_tensor("cfp8", [64, PPAD + 1], FP8, kind="ExternalInput").ap()
    out_d = nc.dram_tensor("out_row", [1, 320], F32, kind="ExternalOutput").ap()
    with tile.TileContext(nc) as tc:
        _kernel_body(tc, out_d, zt_d, cpack_d, cfp8_d, loop_k=loop_k)
    nc.compile()
    _cache[ckey] = nc
    return nc


def _in_maps(Z, perm_index):
    perm = np.asarray(perm_index, dtype=np.int64).reshape(NUM_PERMS, T)
    key = perm.tobytes()
    if _cache.get("ckey") != key:
        _cache["ckey"] = key
        _cache["ctab"] = _c_table(perm)
        _cache["cpack"] = _consts()
    ctab, cpack = _cache["ctab"], _cache["cpack"]
    Zf = np.asarray(Z, dtype=np.float32)
    in_maps = []
    for c in range(N_CORES):
        zc = Zf[c * B_LOC : (c + 1) * B_LOC]  # [32, 8, 128]
        zt = np.ascontiguousarray(zc.transpose(2, 0, 1).reshape(128, 256))
        in_maps.append(
            {
                "zt": zt.astype(ml_dtypes.bfloat16),
                "cpack": cpack,
                "cfp8": ctab,
            }
        )
    return in_maps


def kernel(Z, perm_index, _trace=False):
    nc = _build()
    in_maps = _in_maps(Z, perm_index)
    res = run_bass_kernel_spmd(
        nc, in_maps, core_ids=list(range(N_CORES)), trace=_trace
    )
    total = np.float64(0.0)
    for r in res.results:
        row = np.asarray(r["out_row"], dtype=np.float64).reshape(320)
        s = row[0:256].reshape(8, B_LOC).sum(axis=0)  # [32] per-batch exp sums
        x0 = row[256:288]
        av = row[288:320]
        total += float(np.sum(np.log(s) + 0.5 * x0 + av))
    out = np.array(total / B, dtype=np.float32)
    if _trace:
        return out, res
    return out
